# revision 1
# baseline (speedup 1.0000x reference)
"""Causal multi-head attention for Trainium2 (Bass/Tile), 8 NeuronCores. v3.

Problem: q,k,v [B=4, H=16, S=2048, d=64] fp32;
         out = softmax(causal_mask(QK^T/sqrt(d))) @ V.
Sharding: 64 (b,h) head-slices, 8 per core (head parallelism, no comms).

Design notes (from HW microbenchmarks):
  - The core enforces an activity/power budget (HAM duty-cycling): with
    several engines hot the PE gets ~50% duty.  So v3 minimizes total work
    and bytes on EVERY engine, not just the PE.
  - qT/kT strips [d, S] load straight from the fp32 inputs with
    dma_start_transpose on a bf16 BITCAST: the high halfword of each
    little-endian fp32 is truncated bf16.  A rearranged out-AP lands the
    high-halfword plane on partitions 64:128; the low plane (garbage) on
    0:64 is never read.  Zero casts, zero scratch traffic, zero PE
    transposes; HBM traffic is the 16MB minimum.
  - exp splits between ACT (hw Exp) and DVE (Schraudolph bf16-bits via
    tensor_scalar mult+add into int16, bitcast to bf16); tri-masks run on
    GPSIMD; PV rides a ones-column in v' to emit softmax sums for free.
  - PE stream is software-pipelined (PV deferred one slot) and head pairs
    are interleaved so the PE never waits on an exp.
  - Output: oT -> bf16 -> PE transpose -> reciprocal * scale -> DMA.
"""

import math
import os

import numpy as np

import concourse.bacc as bacc
import concourse.bass as bass
import concourse.mybir as mybir
from concourse.bass_utils import run_bass_kernel_spmd
from concourse.masks import make_identity, make_upper_triangular
from concourse.tile import TileContext

B, H, S, D = 4, 16, 2048, 64
NCORES = 8
HPC = (B * H) // NCORES  # 8 heads per core
QB = 512                 # q-block (one PSUM bank of fp32)
KC = 128                 # k-chunk
NQB = S // QB            # 4
NT = S // 128            # 16

FP32 = mybir.dt.float32
BF16 = mybir.dt.bfloat16
I16 = mybir.dt.int16

# schraudolph: bf16_bits(exp(x*0.125)) ~= x * SCH_A + SCH_B
SCH_A = 128.0 / math.log(2.0) * 0.125
SCH_B = 16248.5
SCH_FRAC = float(os.environ.get("SCH_FRAC", "0.3"))
POOL_FRAC = float(os.environ.get("POOL_FRAC", "0.0"))
WARM_MM = int(os.environ.get("WARM_MM", "36"))
MASK_POOL = bool(int(os.environ.get("MASK_POOL", "1")))
PEND_DEPTH = int(os.environ.get("PEND_DEPTH", "4"))


def build_program() -> bass.Bass:
    nc = bacc.Bacc(None, target_bir_lowering=False, debug=False)

    q_in = nc.declare_dram_parameter("q", [HPC, S, D], FP32, isOutput=False)
    k_in = nc.declare_dram_parameter("k", [HPC, S, D], FP32, isOutput=False)
    v_in = nc.declare_dram_parameter("v", [HPC, S, D], FP32, isOutput=False)
    out_p = nc.declare_dram_parameter("out", [HPC, S, D], FP32, isOutput=True)

    with TileContext(nc) as tc:
        with (
            tc.tile_pool(name="consts", bufs=1) as consts,
            tc.tile_pool(name="inp", bufs=2) as inp,
            tc.tile_pool(name="strip", bufs=2) as strip,
            tc.tile_pool(name="ppool", bufs=6) as ppool,
            tc.tile_pool(name="osb", bufs=3) as osb,
            tc.tile_pool(name="res", bufs=4) as res,
            tc.tile_pool(name="ps_s", bufs=3, space="PSUM") as ps_s,
            tc.tile_pool(name="ps_o", bufs=2, space="PSUM") as ps_o,
        ):
            ident = consts.tile([128, 128], FP32)
            make_identity(nc, ident)
            identb = consts.tile([128, 128], BF16)
            nc.vector.tensor_copy(identb, ident)
            tri_f32 = consts.tile([128, 128], FP32)
            make_upper_triangular(nc, tri_f32, val=1.0, diag=True)
            tri = consts.tile([128, 128], BF16)
            nc.vector.tensor_copy(tri, tri_f32)
            ones_c = consts.tile([128, NT], FP32)
            nc.vector.memset(ones_c, 1.0)
            warm = consts.tile([1, 8], FP32)
            nc.scalar.activation(
                warm, ones_c[0:1, 0:8], mybir.ActivationFunctionType.Exp
            )
            # PE clock warm-up while the first strips arrive
            wtp = ps_s.tile([128, 2048], BF16, tag="sP", name="wtp")
            for i in range(WARM_MM):
                off = 512 * (i % 4)
                nc.tensor.transpose(wtp[:, off : off + 128], identb, identb)

            # ---------------- prep ---------------------------------------
            def prep_strips(i, st, j):
                """XBAR-transpose head (2i+j)'s q/k straight from DRAM.

                in_: fp32 [S, 64] bitcast to bf16 [S, 128]: halfword column
                c = 2d+h of row s, where h=1 is the truncated-bf16 plane.
                The XBAR lands column c on partition c (interleaved), so a
                second on-chip DMA compacts the odd partitions into the
                shared strip tile: head j=0 -> partitions 0:64, j=1 ->
                64:128.  No casts, no scratch HBM traffic.
                """
                h = 2 * i + j
                for name, src in (("qT", q_in), ("kT", k_in)):
                    raw = strip.tile([128, S], BF16, tag=f"raw{name}{j}",
                                     name=f"raw{name}{j}")
                    t = st.get(name)
                    if t is None:
                        t = strip.tile([128, S], BF16, tag=name, name=name)
                        st[name] = t
                    nc.sync.dma_start_transpose(raw, src[h].bitcast(BF16))
                    nc.sync.dma_start(
                        out=t[64 * j : 64 * (j + 1), :],
                        in_=raw.rearrange("(d h) s -> h d s", h=2)[1],
                    )

            def prep_strips_first(st):
                """Pair 0: half-strip XBARs, interleaved so both heads' b0/b1
                columns (0:1024) arrive first and unblock the PE early."""
                raws = {}
                for name in ("qT", "kT"):
                    st[name] = strip.tile([128, S], BF16, tag=name, name=name)
                    for j in range(2):
                        raws[(name, j)] = strip.tile(
                            [128, S], BF16, tag=f"raw{name}{j}",
                            name=f"raw{name}{j}",
                        )
                for sl in (
                    slice(0, 1024),
                    slice(1024, 2048),
                ):
                    for j in range(2):
                        for name, src in (("qT", q_in), ("kT", k_in)):
                            raw = raws[(name, j)]
                            nc.sync.dma_start_transpose(
                                raw[:, sl], src[2 * 0 + j].bitcast(BF16)[sl, :]
                            )
                            nc.sync.dma_start(
                                out=st[name][64 * j : 64 * (j + 1), sl],
                                in_=raw.rearrange("(d h) s -> h d s", h=2)[1][
                                    :, sl
                                ],
                            )

            def prep_loads_v(i, st):
                # gpsimd swdge queue: parallel with the strip XBARs on sync
                for j, h in enumerate((2 * i, 2 * i + 1)):
                    vsb = inp.tile([128, NT, D], FP32, tag="vsb", name=f"vsb{j}")
                    nc.gpsimd.dma_start(
                        out=vsb, in_=v_in[h].rearrange("(t p) d -> p t d", p=128)
                    )
                    st[f"vsb{j}"] = vsb

            def prep_cast_v(i, st):
                for j in range(2):
                    vb = inp.tile(
                        [128, NT, D + 1], BF16, tag="vb", name=f"vb{j}", bufs=4
                    )
                    nc.vector.tensor_copy(vb[:, :, 0:D], st[f"vsb{j}"])
                    nc.vector.tensor_copy(vb[:, :, D], ones_c)
                    st[f"vb{j}"] = vb

            # ---------------- main loop pieces ---------------------------
            sch_state = [0.0, 0.0, 0.0]  # [total_cols, dve_cols, pool_cols]

            def pick_producer(cols):
                sch_state[0] += cols
                if sch_state[2] < POOL_FRAC * sch_state[0]:
                    sch_state[2] += cols
                    return "pool"
                if sch_state[1] < SCH_FRAC * sch_state[0]:
                    sch_state[1] += cols
                    return "dve"
                return "act"

            def emit_qk(st, j, b, m):
                qT, kT = st["qT"], st["kT"]
                o = 64 * j
                cs = (2 * m, 2 * m + 1)
                ts = [c - 4 * b for c in cs]
                j0s = [128 * t if t >= 0 else 0 for t in ts]
                sP = ps_s.tile([128, 2, QB], FP32, tag="sP", name="sP")
                for x in range(2):
                    nc.tensor.matmul(
                        sP[:, x, j0s[x] : QB],
                        kT[o : o + 64, KC * cs[x] : KC * (cs[x] + 1)],
                        qT[o : o + 64, QB * b + j0s[x] : QB * (b + 1)],
                        start=True,
                        stop=True,
                    )
                return sP, j0s, ts

            def emit_exp(sP, j0s, ts):
                pTi = ppool.tile([128, 2, QB], I16, tag="pT", name="pT")
                pT = pTi.bitcast(BF16)
                sPf = sP.rearrange("p a f -> p (a f)")
                pTf = pTi.rearrange("p a f -> p (a f)")
                pTfb = pT.rearrange("p a f -> p (a f)")
                def emit_one(dst_bf, dst_i16, src, cols):
                    # GPSIMD cannot read PSUM, so producers are ACT/DVE only
                    prod = pick_producer(cols)
                    if prod == "act":
                        nc.scalar.activation(
                            dst_bf, src,
                            mybir.ActivationFunctionType.Exp, scale=0.125,
                        )
                    else:
                        nc.vector.tensor_scalar(
                            dst_i16, src, SCH_A, SCH_B,
                            mybir.AluOpType.mult, mybir.AluOpType.add,
                        )

                if j0s[0] == 0:
                    emit_one(
                        pTfb[:, 0 : 2 * QB], pTf[:, 0 : 2 * QB],
                        sPf[:, 0 : 2 * QB], 2 * QB,
                    )
                else:
                    for x in range(2):
                        emit_one(
                            pT[:, x, j0s[x] : QB], pTi[:, x, j0s[x] : QB],
                            sP[:, x, j0s[x] : QB], QB - j0s[x],
                        )
                eng = nc.gpsimd if MASK_POOL else nc.vector
                for x in range(2):
                    if ts[x] >= 0:
                        eng.tensor_mul(
                            pT[:, x, j0s[x] : j0s[x] + 128],
                            pT[:, x, j0s[x] : j0s[x] + 128],
                            tri,
                        )
                return pT

            def emit_pv(vb, oT, pT, j0s, m, npairs):
                cs = (2 * m, 2 * m + 1)
                for x in range(2):
                    nc.tensor.matmul(
                        oT[:, j0s[x] : QB],
                        vb[:, cs[x]],
                        pT[:, x, j0s[x] : QB],
                        start=(m == 0 and x == 0),
                        stop=(m == npairs - 1 and x == 1),
                    )

            def emit_output(h, b, oT):
                oTc = osb.tile([D + 1, QB], BF16, name="oTc")
                nc.vector.tensor_copy(oTc, oT)
                otr = ps_s.tile(
                    [128, 4, D + 1], BF16, tag="sP", name="otr",
                    padded_shape=[128, 4, 512],
                )
                for i in range(4):
                    nc.tensor.transpose(
                        otr[:, i],
                        oTc[:, 128 * i : 128 * (i + 1)],
                        identb[0 : D + 1, 0 : D + 1],
                    )
                rec = res.tile([128, 4], FP32, name="rec")
                nc.vector.reciprocal(rec, otr[:, :, D])
                ores = res.tile([128, 4, D], FP32, name="ores")
                for i in range(4):
                    nc.vector.tensor_scalar_mul(
                        ores[:, i], otr[:, i, 0:D], rec[:, i : i + 1]
                    )
                nc.sync.dma_start(
                    out=out_p[h, QB * b : QB * (b + 1), :].rearrange(
                        "(t p) d -> p t d", p=128
                    ),
                    in_=ores,
                )

            # ---------------- schedule -----------------------------------
            NP = HPC // 2
            st_cur = {}
            prep_loads_v(0, st_cur)
            prep_strips_first(st_cur)
            prep_cast_v(0, st_cur)

            deferred_prev = []
            pend = []  # (vb, oT, pT, j0s, m, npairs): PV deferred one slot
            for i in range(NP):
                st_nxt = {} if i + 1 < NP else None
                deferred = []
                for b in range(NQB):
                    npairs = 2 * (b + 1)
                    oTs = [
                        ps_o.tile([D + 1, QB], FP32, tag="oT", name=f"oT{j}")
                        for j in range(2)
                    ]
                    gb = i * NQB + b
                    order = [(m, j) for m in range(npairs) for j in range(2)]
                    for m, j in order:
                        sP, j0s, ts = emit_qk(st_cur, j, b, m)
                        pT = emit_exp(sP, j0s, ts)
                        pend.append(
                            (st_cur[f"vb{j}"], oTs[j], pT, j0s, m, npairs, gb)
                        )
                        if len(pend) > PEND_DEPTH:
                            emit_pv(*pend.pop(0)[:6])
                        if m <= 1 and j == 1 and (deferred_prev or deferred):
                            # stagger the two heads' output stages (m=0 and
                            # m=1) so only one otr occupies an sP slot at a
                            # time; drain the previous block's deferred PVs
                            # first or the in-order PE queue deadlocks
                            while pend and pend[0][6] < gb:
                                emit_pv(*pend.pop(0)[:6])
                            todo = deferred_prev + deferred
                            deferred_prev = []
                            deferred = []
                            emit_output(*todo[0])
                            if m == 1 or npairs == 2:
                                for args in todo[1:]:
                                    emit_output(*args)
                            else:
                                deferred = todo[1:]
                    # prep interleave points at block boundaries
                    if st_nxt is not None:
                        if b == 0:
                            prep_strips(i + 1, st_nxt, 0)
                        elif b == 1:
                            prep_loads_v(i + 1, st_nxt)
                            prep_strips(i + 1, st_nxt, 1)
                        elif b == 2:
                            prep_cast_v(i + 1, st_nxt)
                    deferred = [(2 * i + j, b, oTs[j]) for j in range(2)]
                deferred_prev = deferred
                st_cur = st_nxt
            while pend:
                emit_pv(*pend.pop(0)[:6])
            for args in deferred_prev:
                emit_output(*args)

    nc.compile()
    return nc


_NC_CACHE = None
LAST_RESULT = None


def kernel(q: np.ndarray, k: np.ndarray, v: np.ndarray) -> np.ndarray:
    global _NC_CACHE, LAST_RESULT
    if _NC_CACHE is None:
        _NC_CACHE = build_program()
    nc = _NC_CACHE

    def shard(x):
        x = np.ascontiguousarray(np.asarray(x, dtype=np.float32)).reshape(
            B * H, S, D
        )
        return [
            np.ascontiguousarray(x[i * HPC : (i + 1) * HPC])
            for i in range(NCORES)
        ]

    qs, ks, vs = shard(q), shard(k), shard(v)
    ncores = int(os.environ.get("KCORES", str(NCORES)))
    in_maps = [{"q": qs[i], "k": ks[i], "v": vs[i]} for i in range(NCORES)]
    trace = bool(int(os.environ.get("KERNEL_TRACE", "0")))
    result = run_bass_kernel_spmd(
        nc, in_maps[:ncores], core_ids=list(range(ncores)), trace=trace
    )
    LAST_RESULT = result
    outs = [r["out"] for r in result.results]
    if ncores < NCORES:
        outs += [np.zeros((HPC, S, D), np.float32)] * (NCORES - ncores)
    out = np.concatenate(outs, axis=0)
    return out.reshape(B, H, S, D)

